# Initial kernel scaffold
#
"""Trainium2 Bass kernel for nn_ClassifyMLPHeadForKCRWithConcatChoices.

Math (B=16, L=2048, H=A=1024, C=5):
  keys  = tanh(X @ Wh^T + bh)                    (B,L,A)
  probs = keys @ (q / sqrt(A*var(q)))            (B,L)
  z     = probs * (-1000 * (1 - attn))           (B,L)
  att   = softmax_L(z)                           (B,L)
  vals  = att[...,None] + X                      (B,L,H)
  ctx   = einsum('bcl,blh->bch', seg, vals)
  logit = ctx @ Wc^T + bc                        (B,C,1)

Because att broadcasts over H and the classifier is rank-1:
  logit[b,c] = (seg·att)[b,c] * sum(Wc) + (seg·y)[b,c] + bc,  y = X @ Wc
so the device only computes the heavy parts — keys/probs (68.7 GFLOP matmul +
tanh), the per-row softmax, and the per-token classifier projection y — and
returns per-token att and y.  The O(B*C*L) segment pooling runs on the host
during unsharding.

Sharding: data-parallel over batch, 2 rows per core x 8 cores; weights
replicated.  X is pre-transposed on the host to (H, tokens) so the contraction
dim lies on SBUF partitions, and cast to bf16 (PE fp32 matmul is 4.5x slower;
validated end-to-end rel err ~2e-3).
"""

import sys

if '/opt/trn_rl_repo' not in sys.path:
    sys.path.insert(0, '/opt/trn_rl_repo')

import numpy as np
import ml_dtypes

import concourse.bass as bass  # noqa: F401  (bass must import before bacc)
import concourse.mybir as mybir
import concourse.tile as tile
from concourse import bacc
from concourse.bass_utils import run_bass_kernel_spmd

B, L, H, A, C = 16, 2048, 1024, 1024, 5
N_CORES = 8
RPC = B // N_CORES          # batch rows per core
NTOK = RPC * L              # tokens per core
P = 128
HB, AB = H // P, A // P     # contraction / output blocks
CH = 512                    # token chunk (one PSUM bank)
NCH = NTOK // CH

BF16 = mybir.dt.bfloat16
FP32 = mybir.dt.float32


def build_program(repeat: int = 1):
    nc = bacc.Bacc("TRN2", target_bir_lowering=False, debug=False,
                   num_devices=N_CORES)
    xt_d = nc.dram_tensor("xt", [HB, P, NTOK], BF16, kind="ExternalInput")
    wht_d = nc.dram_tensor("wht", [HB, P, A], BF16, kind="ExternalInput")
    qs_d = nc.dram_tensor("qs", [P, AB], BF16, kind="ExternalInput")
    wc_d = nc.dram_tensor("wc", [P, HB], BF16, kind="ExternalInput")
    bh_d = nc.dram_tensor("bh", [P, AB], FP32, kind="ExternalInput")
    mm_d = nc.dram_tensor("mm", [1, NTOK], FP32, kind="ExternalInput")
    out_d = nc.dram_tensor("out", [2, NTOK], FP32, kind="ExternalOutput")

    with tile.TileContext(nc) as tc:
        with (
            tc.tile_pool(name="const", bufs=1) as const,
            tc.tile_pool(name="xpool", bufs=1) as xpool,
            tc.tile_pool(name="keys", bufs=3) as keys,
            tc.tile_pool(name="vecs", bufs=1) as vecs,
            tc.tile_pool(name="ps_k", bufs=2, space="PSUM") as ps_k,
            tc.tile_pool(name="ps_s", bufs=4, space="PSUM") as ps_s,
        ):
            wht_sb = const.tile([P, HB, A], BF16)
            for hb in range(HB):
                nc.sync.dma_start(wht_sb[:, hb, :], wht_d.ap()[hb])
            qs_sb = const.tile([P, AB], BF16)
            nc.sync.dma_start(qs_sb[:], qs_d.ap())
            wc_sb = const.tile([P, HB], BF16)
            nc.sync.dma_start(wc_sb[:], wc_d.ap())
            bh_sb = const.tile([P, AB], FP32)
            nc.sync.dma_start(bh_sb[:], bh_d.ap())
            mm_sb = const.tile([1, NTOK], FP32)
            nc.sync.dma_start(mm_sb[:], mm_d.ap())

            # X^T staged per (hb, chunk) so compute can start after the first
            # column of h-blocks lands.
            xt_sb = {}
            for ch in range(NCH):
                for hb in range(HB):
                    t = xpool.tile([P, CH], BF16, tag=f"x{hb}_{ch}")
                    nc.sync.dma_start(
                        t[:], xt_d.ap()[hb, :, ch * CH:(ch + 1) * CH])
                    xt_sb[hb, ch] = t

            for _ in range(repeat):
                probs_sb = vecs.tile([1, NTOK], FP32, tag="probs")
                y_sb = vecs.tile([1, NTOK], FP32, tag="y")
                for ch in range(NCH):
                    pprobs = ps_s.tile([1, CH], FP32, tag="pprobs")
                    for ab in range(AB):
                        pk = ps_k.tile([P, CH], FP32, tag="pk")
                        for hb in range(HB):
                            nc.tensor.matmul(
                                pk[:],
                                lhsT=wht_sb[:, hb, ab * P:(ab + 1) * P],
                                rhs=xt_sb[hb, ch][:],
                                start=(hb == 0), stop=(hb == HB - 1),
                            )
                        ks = keys.tile([P, CH], BF16, tag="ks")
                        nc.scalar.activation(
                            ks[:], pk[:], mybir.ActivationFunctionType.Tanh,
                            bias=bh_sb[:, ab:ab + 1], scale=1.0)
                        nc.tensor.matmul(
                            pprobs[:], lhsT=qs_sb[:, ab:ab + 1], rhs=ks[:],
                            start=(ab == 0), stop=(ab == AB - 1))
                    nc.vector.tensor_copy(
                        probs_sb[:, ch * CH:(ch + 1) * CH], pprobs[:])
                    py = ps_s.tile([1, CH], FP32, tag="py")
                    for hb in range(HB):
                        nc.tensor.matmul(
                            py[:], lhsT=wc_sb[:, hb:hb + 1],
                            rhs=xt_sb[hb, ch][:],
                            start=(hb == 0), stop=(hb == HB - 1))
                    nc.vector.tensor_copy(
                        y_sb[:, ch * CH:(ch + 1) * CH], py[:])

                # softmax over each batch row's L tokens
                z_sb = vecs.tile([1, NTOK], FP32, tag="z")
                nc.vector.tensor_mul(z_sb[:], probs_sb[:], mm_sb[:])
                att_sb = vecs.tile([1, NTOK], FP32, tag="att")
                for r in range(RPC):
                    sl = slice(r * L, (r + 1) * L)
                    zmax = vecs.tile([1, 1], FP32, tag=f"zmax{r}")
                    nc.vector.reduce_max(zmax[:], z_sb[:, sl],
                                         axis=mybir.AxisListType.X)
                    negmax = vecs.tile([1, 1], FP32, tag=f"negmax{r}")
                    nc.scalar.mul(negmax[:], zmax[:], -1.0)
                    e_sb = vecs.tile([1, L], FP32, tag=f"e{r}")
                    nc.scalar.activation(
                        e_sb[:], z_sb[:, sl], mybir.ActivationFunctionType.Exp,
                        bias=negmax[:], scale=1.0)
                    zsum = vecs.tile([1, 1], FP32, tag=f"zsum{r}")
                    nc.vector.reduce_sum(zsum[:], e_sb[:],
                                         axis=mybir.AxisListType.X)
                    rz = vecs.tile([1, 1], FP32, tag=f"rz{r}")
                    nc.vector.reciprocal(rz[:], zsum[:])
                    nc.vector.tensor_scalar_mul(att_sb[:, sl], e_sb[:],
                                                scalar1=rz[:])
                nc.sync.dma_start(out_d.ap()[0:1, :], att_sb[:])
                nc.sync.dma_start(out_d.ap()[1:2, :], y_sb[:])

    nc.compile()
    return nc


def prep_inputs(inputs):
    """Full inputs -> (per-core in_maps, host epilogue constants)."""
    X = np.ascontiguousarray(np.asarray(inputs["input"], dtype=np.float32))
    attn = np.asarray(inputs["attention_mask"])
    mlm = np.asarray(inputs["mlm_mask"])
    Wh = np.asarray(inputs["W_hidden"], dtype=np.float32)
    bh = np.asarray(inputs["b_hidden"], dtype=np.float32)
    q = np.asarray(inputs["query"], dtype=np.float32)[:, 0]
    Wc = np.asarray(inputs["W_cls"], dtype=np.float32)[0]
    bc = float(np.asarray(inputs["b_cls"], dtype=np.float32)[0])

    qvar = np.var(q.astype(np.float64), ddof=1)
    scale = 1.0 / np.sqrt(A * qvar)

    wht = np.ascontiguousarray(Wh.T).reshape(HB, P, A).astype(ml_dtypes.bfloat16)
    qs = np.ascontiguousarray(
        (q * scale).reshape(AB, P).T).astype(ml_dtypes.bfloat16)
    wc = np.ascontiguousarray(Wc.reshape(HB, P).T).astype(ml_dtypes.bfloat16)
    bh_a = np.ascontiguousarray(bh.reshape(AB, P).T).astype(np.float32)
    maskmul = ((1.0 - attn.astype(np.float32)) * -1000.0)

    XT = X.reshape(B * L, H).T  # (H, B*L) view
    in_maps = []
    for c in range(N_CORES):
        xt_c = np.ascontiguousarray(
            XT[:, c * NTOK:(c + 1) * NTOK]).reshape(HB, P, NTOK)
        in_maps.append(dict(
            xt=xt_c.astype(ml_dtypes.bfloat16),
            wht=wht, qs=qs, wc=wc, bh=bh_a,
            mm=np.ascontiguousarray(
                maskmul.reshape(1, B * L)[:, c * NTOK:(c + 1) * NTOK]),
        ))
    return in_maps, (attn, mlm, Wc, bc)


def epilogue(att, y, attn, mlm, Wc, bc):
    """Segment pooling + rank-1 classifier on host.  att/y: (B, L) fp32."""
    idx = np.arange(L)
    marker = np.where(mlm > 0, idx[None, :], L)
    starts = np.sort(marker, axis=1)[:, :C]
    end_idx = attn.sum(axis=1)
    bounds = np.concatenate([starts[:, 1:] - 1, (end_idx - 1)[:, None]], axis=1)
    seg = ((idx[None, None, :] >= starts[:, :, None] + 1)
           & (idx[None, None, :] < bounds[:, :, None])).astype(np.float32)
    S_att = np.einsum("bcl,bl->bc", seg, att)
    Sy = np.einsum("bcl,bl->bc", seg, y)
    Wsum = Wc.sum(dtype=np.float32)
    return (S_att * Wsum + Sy + bc).astype(np.float32)[:, :, None]


_prog_cache = {}


def kernel(**inputs) -> np.ndarray:
    if "prog" not in _prog_cache:
        _prog_cache["prog"] = build_program()
    nc = _prog_cache["prog"]
    in_maps, (attn, mlm, Wc, bc) = prep_inputs(inputs)
    res = run_bass_kernel_spmd(nc, in_maps, core_ids=list(range(N_CORES)))
    att = np.concatenate(
        [res.results[c]["out"][0].reshape(RPC, L) for c in range(N_CORES)])
    y = np.concatenate(
        [res.results[c]["out"][1].reshape(RPC, L) for c in range(N_CORES)])
    return epilogue(att, y, attn, mlm, Wc, bc)


# revision 3
# speedup vs baseline: 1.6566x; 1.6566x over previous
"""Trainium2 Bass kernel for nn_ClassifyMLPHeadForKCRWithConcatChoices.

Math (B=16, L=2048, H=A=1024, C=5):
  keys  = tanh(X @ Wh^T + bh)                    (B,L,A)
  probs = keys @ (q / sqrt(A*var(q)))            (B,L)
  z     = probs * (-1000 * (1 - attn))           (B,L)
  att   = softmax_L(z)                           (B,L)
  vals  = att[...,None] + X                      (B,L,H)
  ctx   = einsum('bcl,blh->bch', seg, vals)
  logit = ctx @ Wc^T + bc                        (B,C,1)

Because att broadcasts over H and the classifier is rank-1:
  logit[b,c] = (seg·att)[b,c] * sum(Wc) + (seg·y)[b,c] + bc,  y = X @ Wc
so the device only computes the heavy parts — keys/probs (68.7 GFLOP matmul +
tanh), the per-row softmax, and the per-token classifier projection y — and
returns per-token att and y.  The O(B*C*L) segment pooling runs on the host
during unsharding.

Sharding: data-parallel over batch, 2 rows per core x 8 cores; weights
replicated.  X is pre-transposed on the host to (H, tokens) so the contraction
dim lies on SBUF partitions, and cast to bf16 (PE fp32 matmul is 4.5x slower;
validated end-to-end rel err ~2e-3).
"""

import sys

if '/opt/trn_rl_repo' not in sys.path:
    sys.path.insert(0, '/opt/trn_rl_repo')

import numpy as np
import ml_dtypes

import concourse.bass as bass  # noqa: F401  (bass must import before bacc)
import concourse.mybir as mybir
import concourse.tile as tile
from concourse import bacc
from concourse.bass_utils import run_bass_kernel_spmd

B, L, H, A, C = 16, 2048, 1024, 1024, 5
N_CORES = 8
RPC = B // N_CORES          # batch rows per core
NTOK = RPC * L              # tokens per core
P = 128
HB, AB = H // P, A // P     # contraction / output blocks
CH = 512                    # token chunk (one PSUM bank)
NCH = NTOK // CH

BF16 = mybir.dt.bfloat16
FP32 = mybir.dt.float32


def build_program(repeat: int = 1, n_cores: int = N_CORES):
    nc = bacc.Bacc("TRN2", target_bir_lowering=False, debug=False,
                   num_devices=n_cores)
    xt_d = nc.dram_tensor("xt", [HB, P, NTOK], BF16, kind="ExternalInput")
    wht_d = nc.dram_tensor("wht", [HB, P, A], BF16, kind="ExternalInput")
    qs_d = nc.dram_tensor("qs", [P, AB], BF16, kind="ExternalInput")
    wc_d = nc.dram_tensor("wc", [P, HB], BF16, kind="ExternalInput")
    bh_d = nc.dram_tensor("bh", [P, AB], FP32, kind="ExternalInput")
    mm_d = nc.dram_tensor("mm", [1, NTOK], FP32, kind="ExternalInput")
    out_d = nc.dram_tensor("out", [2, NTOK], FP32, kind="ExternalOutput")

    with tile.TileContext(nc) as tc:
        with (
            tc.tile_pool(name="const", bufs=1) as const,
            tc.tile_pool(name="xpool", bufs=1) as xpool,
            tc.tile_pool(name="keys", bufs=3) as keys,
            tc.tile_pool(name="vecs", bufs=1) as vecs,
            tc.tile_pool(name="ps_k", bufs=2, space="PSUM") as ps_k,
            tc.tile_pool(name="ps_s", bufs=2, space="PSUM") as ps_s,
        ):
            wht_sb = const.tile([P, HB, A], BF16)
            for hb in range(HB):
                nc.sync.dma_start(wht_sb[:, hb, :], wht_d.ap()[hb])
            qs_sb = const.tile([P, AB], BF16)
            nc.sync.dma_start(qs_sb[:], qs_d.ap())
            wc_sb = const.tile([P, HB], BF16)
            nc.sync.dma_start(wc_sb[:], wc_d.ap())
            bh_sb = const.tile([P, AB], FP32)
            nc.sync.dma_start(bh_sb[:], bh_d.ap())
            mm_sb = const.tile([1, NTOK], FP32)
            nc.sync.dma_start(mm_sb[:], mm_d.ap())

            # X^T staged per (hb, chunk) so compute can start after the first
            # column of h-blocks lands.
            xt_sb = {}
            for ch in range(NCH):
                for hb in range(HB):
                    t = xpool.tile([P, CH], BF16, tag=f"x{hb}_{ch}")
                    nc.sync.dma_start(
                        t[:], xt_d.ap()[hb, :, ch * CH:(ch + 1) * CH])
                    xt_sb[hb, ch] = t

            for _ in range(repeat):
                probs_sb = vecs.tile([1, NTOK], FP32, tag="probs")
                y_sb = vecs.tile([1, NTOK], FP32, tag="y")
                for ch in range(NCH):
                    pprobs = ps_s.tile([1, CH], FP32, tag="pprobs")
                    for ab in range(AB):
                        pk = ps_k.tile([P, CH], FP32, tag="pk")
                        for hb in range(HB):
                            nc.tensor.matmul(
                                pk[:],
                                lhsT=wht_sb[:, hb, ab * P:(ab + 1) * P],
                                rhs=xt_sb[hb, ch][:],
                                start=(hb == 0), stop=(hb == HB - 1),
                            )
                        ks = keys.tile([P, CH], BF16, tag="ks")
                        nc.scalar.activation(
                            ks[:], pk[:], mybir.ActivationFunctionType.Tanh,
                            bias=bh_sb[:, ab:ab + 1], scale=1.0)
                        nc.tensor.matmul(
                            pprobs[:], lhsT=qs_sb[:, ab:ab + 1], rhs=ks[:],
                            start=(ab == 0), stop=(ab == AB - 1))
                    nc.vector.tensor_copy(
                        probs_sb[:, ch * CH:(ch + 1) * CH], pprobs[:])
                    py = ps_s.tile([1, CH], FP32, tag="py")
                    for hb in range(HB):
                        nc.tensor.matmul(
                            py[:], lhsT=wc_sb[:, hb:hb + 1],
                            rhs=xt_sb[hb, ch][:],
                            start=(hb == 0), stop=(hb == HB - 1))
                    nc.vector.tensor_copy(
                        y_sb[:, ch * CH:(ch + 1) * CH], py[:])

                # softmax over each batch row's L tokens
                z_sb = vecs.tile([1, NTOK], FP32, tag="z")
                nc.vector.tensor_mul(z_sb[:], probs_sb[:], mm_sb[:])
                att_sb = vecs.tile([1, NTOK], FP32, tag="att")
                for r in range(RPC):
                    sl = slice(r * L, (r + 1) * L)
                    zmax = vecs.tile([1, 1], FP32, tag=f"zmax{r}")
                    nc.vector.reduce_max(zmax[:], z_sb[:, sl],
                                         axis=mybir.AxisListType.X)
                    negmax = vecs.tile([1, 1], FP32, tag=f"negmax{r}")
                    nc.scalar.mul(negmax[:], zmax[:], -1.0)
                    e_sb = vecs.tile([1, L], FP32, tag=f"e{r}")
                    nc.scalar.activation(
                        e_sb[:], z_sb[:, sl], mybir.ActivationFunctionType.Exp,
                        bias=negmax[:], scale=1.0)
                    zsum = vecs.tile([1, 1], FP32, tag=f"zsum{r}")
                    nc.vector.reduce_sum(zsum[:], e_sb[:],
                                         axis=mybir.AxisListType.X)
                    rz = vecs.tile([1, 1], FP32, tag=f"rz{r}")
                    nc.vector.reciprocal(rz[:], zsum[:])
                    nc.vector.tensor_scalar_mul(att_sb[:, sl], e_sb[:],
                                                scalar1=rz[:])
                nc.sync.dma_start(out_d.ap()[0:1, :], att_sb[:])
                nc.sync.dma_start(out_d.ap()[1:2, :], y_sb[:])

    nc.compile()
    return nc


def prep_inputs(inputs):
    """Full inputs -> (per-core in_maps, host epilogue constants)."""
    X = np.ascontiguousarray(np.asarray(inputs["input"], dtype=np.float32))
    attn = np.asarray(inputs["attention_mask"])
    mlm = np.asarray(inputs["mlm_mask"])
    Wh = np.asarray(inputs["W_hidden"], dtype=np.float32)
    bh = np.asarray(inputs["b_hidden"], dtype=np.float32)
    q = np.asarray(inputs["query"], dtype=np.float32)[:, 0]
    Wc = np.asarray(inputs["W_cls"], dtype=np.float32)[0]
    bc = float(np.asarray(inputs["b_cls"], dtype=np.float32)[0])

    qvar = np.var(q.astype(np.float64), ddof=1)
    scale = 1.0 / np.sqrt(A * qvar)

    wht = np.ascontiguousarray(Wh.T).reshape(HB, P, A).astype(ml_dtypes.bfloat16)
    qs = np.ascontiguousarray(
        (q * scale).reshape(AB, P).T).astype(ml_dtypes.bfloat16)
    wc = np.ascontiguousarray(Wc.reshape(HB, P).T).astype(ml_dtypes.bfloat16)
    bh_a = np.ascontiguousarray(bh.reshape(AB, P).T).astype(np.float32)
    maskmul = ((1.0 - attn.astype(np.float32)) * -1000.0)

    XT = X.reshape(B * L, H).T  # (H, B*L) view
    in_maps = []
    for c in range(N_CORES):
        xt_c = np.ascontiguousarray(
            XT[:, c * NTOK:(c + 1) * NTOK]).reshape(HB, P, NTOK)
        in_maps.append(dict(
            xt=xt_c.astype(ml_dtypes.bfloat16),
            wht=wht, qs=qs, wc=wc, bh=bh_a,
            mm=np.ascontiguousarray(
                maskmul.reshape(1, B * L)[:, c * NTOK:(c + 1) * NTOK]),
        ))
    return in_maps, (attn, mlm, Wc, bc)


def epilogue(att, y, attn, mlm, Wc, bc):
    """Segment pooling + rank-1 classifier on host.  att/y: (B, L) fp32."""
    idx = np.arange(L)
    marker = np.where(mlm > 0, idx[None, :], L)
    starts = np.sort(marker, axis=1)[:, :C]
    end_idx = attn.sum(axis=1)
    bounds = np.concatenate([starts[:, 1:] - 1, (end_idx - 1)[:, None]], axis=1)
    seg = ((idx[None, None, :] >= starts[:, :, None] + 1)
           & (idx[None, None, :] < bounds[:, :, None])).astype(np.float32)
    S_att = np.einsum("bcl,bl->bc", seg, att)
    Sy = np.einsum("bcl,bl->bc", seg, y)
    Wsum = Wc.sum(dtype=np.float32)
    return (S_att * Wsum + Sy + bc).astype(np.float32)[:, :, None]


_prog_cache = {}


def kernel(**inputs) -> np.ndarray:
    if "prog" not in _prog_cache:
        _prog_cache["prog"] = build_program()
    nc = _prog_cache["prog"]
    in_maps, (attn, mlm, Wc, bc) = prep_inputs(inputs)
    res = run_bass_kernel_spmd(nc, in_maps, core_ids=list(range(N_CORES)))
    att = np.concatenate(
        [res.results[c]["out"][0].reshape(RPC, L) for c in range(N_CORES)])
    y = np.concatenate(
        [res.results[c]["out"][1].reshape(RPC, L) for c in range(N_CORES)])
    return epilogue(att, y, attn, mlm, Wc, bc)
